# revision 16
# baseline (speedup 1.0000x reference)
"""Trainium2 Bass kernel for ConvPixelToCapsules (conv -> 3-iter dynamic routing).

Strategy (hardcoded for x[8,32,8,32,32], conv_w[256,8,3,3], bias[32,8,1,1]):
  - Host precomputes im2col patches per batch element, with an extra 33rd
    "channel" slot holding sum_ci(x) (conv linearity gives iteration-1's
    uniform-route preactivation for free), plus the weight matrix in
    [72, (no,co)] layout and a partition-broadcast bias tile.
  - 8 NeuronCores, data-parallel over batch: core k owns batch element k.
  - Per core: 8 tiles of 128 output pixels. Votes live in SBUF as
    [pixel-partition; (ci,no,co)] bf16; all products are DVE bf16 2x ops.
  - v3: the ci-contraction (preactivation S = sum_ci R*U) runs on the PE as
    32 accumulating identity-matmuls (exact f32 PSUM accumulation, bias
    folded in as a rank-1 matmul), transposed back by the PE. The no-
    contraction (distances) stays a DVE halving tree with the final level
    on GPSIMD. Squash computes sqrt via Quake-rsqrt + Newton on DVE
    (bitcast/shift ops), so the Act engine only ever needs Copy+Exp: one
    activation-table load for the whole program instead of 43.
  - PSUM conv evacuation is paired ([128,512] per copy) to halve Act time.
"""

import numpy as np

BS, CI, NI, H, W = 8, 32, 8, 32, 32
CO, NO = 32, 8
NPIX = H * W            # 1024
TILES = 8               # tiles of 128 pixels per batch element
TP = 128                # pixels per tile (on partitions)
K = 72                  # ni * 3 * 3 contraction
SLOTS = CI + 1          # 32 ci + xsum slot
OUTCH = NO * CO         # 256, (no, co) order
QK = 0x5F3759DF         # Quake rsqrt seed constant

CFG = {
    "pair": True,          # interleave emission of tile pairs
    "newton12": 1,         # Newton iterations for squash 1-2
    "newton3": 2,          # Newton iterations for final squash
    "bias_mm": True,       # fold bias add into the PE accumulation group
    "big_bufs": 3,
    "pconv_bufs": 2,
}

_BUILT = {}


def _host_prep(x, conv_w, bias):
    x = np.asarray(x, np.float32)
    conv_w = np.asarray(conv_w, np.float32)
    bias = np.asarray(bias, np.float32)
    x_pad = np.pad(x, ((0, 0), (0, 0), (0, 0), (1, 1), (1, 1)))
    x_aug = np.concatenate([x_pad, x_pad.sum(1, keepdims=True)], axis=1)
    wv = np.lib.stride_tricks.sliding_window_view(x_aug, (3, 3), axis=(3, 4))
    import ml_dtypes
    cdt_np = ml_dtypes.bfloat16
    patches = np.ascontiguousarray(
        wv.transpose(0, 2, 5, 6, 1, 3, 4).reshape(BS, K, SLOTS, NPIX)
    ).astype(cdt_np)
    w_m = np.ascontiguousarray(
        conv_w.reshape(CO, NO, NI, 3, 3).transpose(2, 3, 4, 1, 0).reshape(K, OUTCH)
    ).astype(cdt_np)
    bias_bc = np.broadcast_to(
        bias[:, :, 0, 0].T.reshape(1, OUTCH), (128, OUTCH)
    ).astype(np.float32)
    ident = np.eye(128, dtype=np.float32)
    identb = np.eye(128, dtype=cdt_np)
    return patches, w_m, bias_bc, ident, identb


def _build_nc():
    key = ("nc",) + tuple(sorted(CFG.items()))
    if key in _BUILT:
        return _BUILT[key]
    import concourse.bacc as bacc
    import concourse.tile as tile
    import concourse.mybir as mybir

    f32 = mybir.dt.float32
    bf16 = mybir.dt.bfloat16
    u32 = mybir.dt.uint32
    AF = mybir.ActivationFunctionType
    OP = mybir.AluOpType
    AX = mybir.AxisListType

    nc = bacc.Bacc("TRN2", target_bir_lowering=False, debug=False, num_devices=8)

    patches_d = nc.dram_tensor("patches", [K, SLOTS, NPIX], bf16, kind="ExternalInput")
    w_d = nc.dram_tensor("w", [K, OUTCH], bf16, kind="ExternalInput")
    bias_d = nc.dram_tensor("bias", [128, OUTCH], f32, kind="ExternalInput")
    ident_d = nc.dram_tensor("ident", [128, 128], f32, kind="ExternalInput")
    identb_d = nc.dram_tensor("identb", [128, 128], bf16, kind="ExternalInput")
    out_d = nc.dram_tensor("out", [2, 128, NPIX], f32, kind="ExternalOutput")

    with tile.TileContext(nc) as tc:
        with (
            tc.tile_pool(name="const", bufs=1) as const,
            tc.tile_pool(name="pat", bufs=3) as patp,
            tc.tile_pool(name="votes", bufs=4) as votesp,
            tc.tile_pool(name="big", bufs=CFG["big_bufs"]) as bigp,
            tc.tile_pool(name="state", bufs=3) as statep,
            tc.tile_pool(name="obuf", bufs=1) as obufp,
            tc.tile_pool(name="pconv", bufs=CFG["pconv_bufs"], space="PSUM") as pconv,
            tc.tile_pool(name="pdist", bufs=2, space="PSUM") as pdist,
            tc.tile_pool(name="ptr", bufs=1, space="PSUM") as ptr,
        ):
            w_sb = const.tile([K, OUTCH], bf16)
            nc.sync.dma_start(w_sb[:], w_d.ap())
            bias_sb = const.tile([128, OUTCH], f32)
            nc.sync.dma_start(bias_sb[:], bias_d.ap())
            ident_sb = const.tile([128, 128], f32)
            nc.sync.dma_start(ident_sb[:], ident_d.ap())
            identb_sb = const.tile([128, 128], bf16)
            nc.sync.dma_start(identb_sb[:], identb_d.ap())
            ones1 = const.tile([1, 128], f32)
            nc.gpsimd.memset(ones1[:], 1.0)
            qc = const.tile([128, CO], u32)
            nc.gpsimd.memset(qc[:], QK)

            ob = [
                obufp.tile([128, NPIX], f32, tag=f"ob{h}", name=f"ob{h}")
                for h in range(2)
            ]

            def conv_tile(t):
                # votes for 128 pixels; Uxs slot first so iteration 1 can
                # start early; ci-pairs share one PSUM bank so the Act
                # evacuation runs half as many, double-width copies.
                pt = patp.tile([K, SLOTS, TP], bf16, tag="pt", name=f"pt{t}")
                nc.sync.dma_start(
                    pt[:, CI, :], patches_d.ap()[:, CI, t * TP : (t + 1) * TP]
                )
                nc.sync.dma_start(
                    pt[:, :CI, :], patches_d.ap()[:, :CI, t * TP : (t + 1) * TP]
                )
                U = votesp.tile([128, CI, NO, CO], bf16, tag="U", name=f"U{t}")
                Uxs = votesp.tile([128, OUTCH], f32, tag="Uxs", name=f"Uxs{t}")
                conv_tile.out[t] = (U, Uxs)
                pvx = pconv.tile([128, 2 * OUTCH], f32, tag="pv", name=f"pvx{t}")
                nc.tensor.matmul(
                    pvx[:, :OUTCH], pt[:, CI, :], w_sb[:], start=True, stop=True
                )
                nc.scalar.copy(Uxs[:], pvx[:, :OUTCH])
                yield
                for c in range(CI // 2):
                    pv = pconv.tile([128, 2 * OUTCH], f32, tag="pv",
                                    name=f"pv{t}_{c}")
                    nc.tensor.matmul(
                        pv[:, :OUTCH], pt[:, 2 * c, :], w_sb[:],
                        start=True, stop=True,
                    )
                    nc.tensor.matmul(
                        pv[:, OUTCH:], pt[:, 2 * c + 1, :], w_sb[:],
                        start=True, stop=True,
                    )
                    dst = U[:, 2 * c : 2 * c + 2].rearrange(
                        "p c n o -> p (c n o)"
                    )
                    nc.scalar.copy(dst, pv[:])
                    yield
            conv_tile.out = {}

            def emit_out(t, V):
                Vf = V[:].rearrange("p n c -> p (n c)")
                for h in range(2):
                    tp = ptr.tile([128, 128], f32, tag="tp", name=f"tp{t}_{h}")
                    nc.tensor.transpose(
                        tp[:], Vf[:, h * 128 : (h + 1) * 128], ident_sb[:]
                    )
                    nc.scalar.copy(ob[h][:, t * TP : (t + 1) * TP], tp[:])
                    nc.sync.dma_start(
                        out_d.ap()[h][:, t * TP : (t + 1) * TP],
                        ob[h][:, t * TP : (t + 1) * TP],
                    )

            def squash(t, S, it, out_dtype, newton):
                # S: [128, NO, CO] f32 (SBUF or PSUM view) -> V [128, NO, CO]
                # scl = sqrt(n)/(1+n) via Quake rsqrt (no act tables needed)
                sq = statep.tile([128, NO, CO], f32, tag="sq", name=f"sq{t}_{it}")
                nc.scalar.square(sq[:], S)
                nsq = statep.tile([128, CO], f32, tag="nsq", name=f"nsq{t}_{it}")
                nc.vector.tensor_reduce(
                    nsq[:], sq[:].transpose([0, 2, 1]), axis=AX.X, op=OP.add
                )
                yield
                sh = statep.tile([128, CO], u32, tag="sh", name=f"sh{t}_{it}")
                nc.vector.tensor_scalar(
                    sh[:], nsq[:].bitcast(u32), 1, None,
                    op0=OP.logical_shift_right,
                )
                y = statep.tile([128, CO], f32, tag="y", name=f"y{t}_{it}")
                nc.vector.tensor_tensor(
                    y[:].bitcast(u32), qc[:], sh[:], op=OP.subtract
                )
                den = statep.tile([128, CO], f32, tag="den", name=f"den{t}_{it}")
                nc.vector.tensor_scalar_add(den[:], nsq[:], 1.0)
                rcd = statep.tile([128, CO], f32, tag="rcd", name=f"rcd{t}_{it}")
                nc.vector.reciprocal(rcd[:], den[:])
                tq = statep.tile([128, CO], f32, tag="tq", name=f"tq{t}_{it}")
                for _ in range(newton):
                    nc.vector.tensor_mul(tq[:], y[:], y[:])
                    nc.vector.tensor_mul(tq[:], tq[:], nsq[:])
                    nc.vector.tensor_scalar(
                        tq[:], tq[:], -0.5, 1.5, op0=OP.mult, op1=OP.add
                    )
                    nc.vector.tensor_mul(y[:], y[:], tq[:])
                yield
                # scl = nsq * y * rcd  (= sqrt(nsq)/(1+nsq))
                scl = statep.tile([128, CO], f32, tag="scl", name=f"scl{t}_{it}")
                nc.vector.tensor_mul(scl[:], nsq[:], y[:])
                nc.vector.tensor_mul(scl[:], scl[:], rcd[:])
                V = statep.tile([128, NO, CO], out_dtype, tag=f"V{it}",
                                name=f"V{t}_{it}")
                nc.vector.tensor_mul(
                    V[:], S, scl[:].unsqueeze(1).broadcast_to([128, NO, CO])
                )
                yield
                squash.out = V

            def s_phase(t, U, R, it):
                # Fused: tmp = U*R (bf16 2x, ci-quarters) pipelined into the
                # PE ci-contraction. Identity stays the stationary, so each
                # matmul is a PSUM-accumulating copy; bias opens the group
                # as a rank-1 ones x bias_row matmul. S lands in [p,(no,co)]
                # f32 PSUM exactly.
                SBt = pdist.tile([128, CI * CO], f32, tag="D",
                                 name=f"SB{t}_{it}")
                SB = SBt[:, :OUTCH]
                tmp = bigp.tile([128, CI, NO, CO], bf16, tag="tmp",
                                name=f"tmps{it}_{t}")
                facb = R[:].unsqueeze(2).broadcast_to([128, CI, NO, CO])
                nc.tensor.matmul(
                    SB, ones1[:], bias_sb[0:1, :],
                    start=True, stop=False, skip_group_check=True,
                )
                Q = CI // 4
                for q in range(4):
                    sl = slice(q * Q, (q + 1) * Q)
                    nc.vector.tensor_mul(tmp[:, sl], U[:, sl], facb[:, sl])
                    yield
                    for ci in range(q * Q, (q + 1) * Q):
                        nc.tensor.matmul(
                            SB, identb_sb[:],
                            tmp[:, ci].rearrange("p n c -> p (n c)"),
                            start=False, stop=(ci == CI - 1),
                            skip_group_check=True,
                        )
                    yield
                s_phase.out = SB

            def d_phase(t, U, V, Dps, it):
                # Fused: tmpn = U*V in no-major layout (strided write keeps
                # co innermost -> DVE 2x survives), pipelined in no-halves
                # into PE accumulating copies: D[p,(ci,co)] = sum_no tmpn.
                # The ci-half h=0 finishes first so the softmax can start
                # on it while h=1 accumulates.
                tmpn = bigp.tile([128, NO, CI, CO], bf16, tag="tmp",
                                 name=f"tmpd{it}_{t}")
                tmp = tmpn[:].transpose([0, 2, 1, 3])
                facb = V[:].unsqueeze(1).broadcast_to([128, CI, NO, CO])
                mvs = [tmpn[:, no].rearrange("p c o -> p (c o)")
                       for no in range(NO)]
                HN = NO // 2
                for g in range(2):
                    nsl = slice(g * HN, (g + 1) * HN)
                    nc.vector.tensor_mul(
                        tmp[:, :, nsl], U[:, :, nsl], facb[:, :, nsl]
                    )
                    yield
                    if g == 0:
                        for h in range(2):
                            for no in range(HN):
                                nc.tensor.matmul(
                                    Dps[:, h * 512 : (h + 1) * 512],
                                    identb_sb[:],
                                    mvs[no][:, h * 512 : (h + 1) * 512],
                                    start=(no == 0), stop=False,
                                    skip_group_check=True,
                                )
                        yield
                    else:
                        for h in range(2):
                            for no in range(HN, NO):
                                nc.tensor.matmul(
                                    Dps[:, h * 512 : (h + 1) * 512],
                                    identb_sb[:],
                                    mvs[no][:, h * 512 : (h + 1) * 512],
                                    start=False, stop=(no == NO - 1),
                                    skip_group_check=True,
                                )
                            yield

            def routing_tile(t, U, Uxs):
                # ---- iteration 1: route is uniform 1/CI ----
                S1 = statep.tile([128, NO, CO], f32, tag="S", name=f"S1_{t}")
                nc.vector.scalar_tensor_tensor(
                    S1[:].rearrange("p n c -> p (n c)"), Uxs[:], 1.0 / CI,
                    bias_sb[:], op0=OP.mult, op1=OP.add,
                )
                yield
                yield from squash(t, S1[:], 1, bf16, CFG["newton12"])
                V1 = squash.out
                Dps = pdist.tile([128, CI * CO], f32, tag="D", name=f"D{t}_1")
                yield from d_phase(t, U, V1, Dps, 1)
                # ---- iterations 2, 3 ----
                V = None
                Eprev = None
                HC = CI // 2
                for it in (2, 3):
                    # softmax over co in ci-halves; logits stay in PSUM
                    # (exact f32). Iteration 3 uses exp(D1+D2) =
                    # exp(D1)*exp(D2) so the distance tile only lives
                    # product -> exp.
                    E = statep.tile([128, CI, CO], bf16, tag="E",
                                    name=f"E{t}_{it}")
                    R = statep.tile([128, CI, CO], bf16, tag="R",
                                    name=f"R{t}_{it}")
                    sume = statep.tile([128, CI], f32, tag="sume",
                                       name=f"sume{t}_{it}")
                    rec = statep.tile([128, CI], f32, tag="rec",
                                      name=f"rec{t}_{it}")
                    for h in range(2):
                        hs = slice(h * HC, (h + 1) * HC)
                        Lv = Dps[:, h * 512 : (h + 1) * 512].rearrange(
                            "p (i c) -> p i c", i=HC
                        )
                        nc.scalar.activation(E[:, hs], Lv, AF.Exp)
                        if it == 3:
                            nc.vector.tensor_mul(
                                E[:, hs], E[:, hs], Eprev[:, hs]
                            )
                        nc.vector.tensor_reduce(
                            sume[:, hs], E[:, hs], axis=AX.X, op=OP.add
                        )
                        nc.vector.reciprocal(rec[:, hs], sume[:, hs])
                        nc.vector.tensor_mul(
                            R[:, hs], E[:, hs],
                            rec[:, hs].unsqueeze(2).broadcast_to(
                                [128, HC, CO]
                            ),
                        )
                        yield
                    Eprev = E
                    yield from s_phase(t, U, R, it)
                    SB = s_phase.out
                    Sv = SB.rearrange("p (n c) -> p n c", n=NO)
                    if it == 2:
                        yield from squash(t, Sv, it, bf16, CFG["newton12"])
                        V = squash.out
                        Dps = pdist.tile([128, CI * CO], f32, tag="D",
                                         name=f"D{t}_2")
                        yield from d_phase(t, U, V, Dps, 2)
                    else:
                        yield from squash(t, Sv, it, f32, CFG["newton3"])
                        V = squash.out
                emit_out(t, V)

            def drain(gens):
                alive = [g for g in gens if g is not None]
                while alive:
                    for g in list(alive):
                        try:
                            next(g)
                        except StopIteration:
                            alive.remove(g)

            if CFG["pair"]:
                groups = [(0, 1), (2, 3), (4, 5), (6, 7)]
                drain([conv_tile(t) for t in groups[0]])
                for gi, grp in enumerate(groups):
                    gens = [routing_tile(t, *conv_tile.out[t]) for t in grp]
                    if gi + 1 < len(groups):
                        gens += [conv_tile(t) for t in groups[gi + 1]]
                    drain(gens)
            else:
                for t in range(TILES):
                    drain([conv_tile(t)])
                    drain([routing_tile(t, *conv_tile.out[t])])

    nc.compile()
    _BUILT[key] = nc
    return nc


def _assemble(out_halves_all):
    o = out_halves_all.reshape(-1, 2, 4, CO, NPIX)
    return np.ascontiguousarray(
        o.transpose(0, 3, 1, 2, 4).reshape(-1, CO, NO, H, W)
    )


def kernel(x, conv_w, bias):
    import sys
    if "/opt/trn_rl_repo" not in sys.path:
        sys.path.insert(0, "/opt/trn_rl_repo")
    from concourse import bass_utils

    patches, w_m, bias_bc, ident, identb = _host_prep(x, conv_w, bias)
    nc = _build_nc()
    in_maps = [
        {"patches": patches[b], "w": w_m, "bias": bias_bc, "ident": ident,
         "identb": identb}
        for b in range(BS)
    ]
    res = bass_utils.run_bass_kernel_spmd(nc, in_maps, core_ids=list(range(BS)))
    outs = np.stack([r["out"] for r in res.results])
    return _assemble(outs).astype(np.float32)


# revision 17
# speedup vs baseline: 1.0180x; 1.0180x over previous
"""Trainium2 Bass kernel for ConvPixelToCapsules (conv -> 3-iter dynamic routing).

Strategy (hardcoded for x[8,32,8,32,32], conv_w[256,8,3,3], bias[32,8,1,1]):
  - Host precomputes im2col patches per batch element, with an extra 33rd
    "channel" slot holding sum_ci(x) (conv linearity gives iteration-1's
    uniform-route preactivation for free), plus the weight matrix in
    [72, (no,co)] layout and a partition-broadcast bias tile.
  - 8 NeuronCores, data-parallel over batch: core k owns batch element k.
  - Per core: 8 tiles of 128 output pixels. Votes live in SBUF as
    [pixel-partition; (ci,no,co)] bf16; all products are DVE bf16 2x ops.
  - v3: the ci-contraction (preactivation S = sum_ci R*U) runs on the PE as
    32 accumulating identity-matmuls (exact f32 PSUM accumulation, bias
    folded in as a rank-1 matmul), transposed back by the PE. The no-
    contraction (distances) stays a DVE halving tree with the final level
    on GPSIMD. Squash computes sqrt via Quake-rsqrt + Newton on DVE
    (bitcast/shift ops), so the Act engine only ever needs Copy+Exp: one
    activation-table load for the whole program instead of 43.
  - PSUM conv evacuation is paired ([128,512] per copy) to halve Act time.
"""

import numpy as np

BS, CI, NI, H, W = 8, 32, 8, 32, 32
CO, NO = 32, 8
NPIX = H * W            # 1024
TILES = 8               # tiles of 128 pixels per batch element
TP = 128                # pixels per tile (on partitions)
K = 72                  # ni * 3 * 3 contraction
SLOTS = CI + 1          # 32 ci + xsum slot
OUTCH = NO * CO         # 256, (no, co) order
QK = 0x5F3759DF         # Quake rsqrt seed constant

CFG = {
    "pair": True,          # interleave emission of tile pairs
    "newton12": 1,         # Newton iterations for squash 1-2
    "newton3": 2,          # Newton iterations for final squash
    "bias_mm": True,       # fold bias add into the PE accumulation group
    "big_bufs": 3,
    "pconv_bufs": 2,
}

_BUILT = {}


def _host_prep(x, conv_w, bias):
    x = np.asarray(x, np.float32)
    conv_w = np.asarray(conv_w, np.float32)
    bias = np.asarray(bias, np.float32)
    x_pad = np.pad(x, ((0, 0), (0, 0), (0, 0), (1, 1), (1, 1)))
    x_aug = np.concatenate([x_pad, x_pad.sum(1, keepdims=True)], axis=1)
    wv = np.lib.stride_tricks.sliding_window_view(x_aug, (3, 3), axis=(3, 4))
    import ml_dtypes
    cdt_np = ml_dtypes.bfloat16
    patches = np.ascontiguousarray(
        wv.transpose(0, 2, 5, 6, 1, 3, 4).reshape(BS, K, SLOTS, NPIX)
    ).astype(cdt_np)
    w_m = np.ascontiguousarray(
        conv_w.reshape(CO, NO, NI, 3, 3).transpose(2, 3, 4, 1, 0).reshape(K, OUTCH)
    ).astype(cdt_np)
    bias_bc = np.broadcast_to(
        bias[:, :, 0, 0].T.reshape(1, OUTCH), (128, OUTCH)
    ).astype(np.float32)
    ident = np.eye(128, dtype=np.float32)
    identb = np.eye(128, dtype=cdt_np)
    return patches, w_m, bias_bc, ident, identb


def _build_nc():
    key = ("nc",) + tuple(sorted(CFG.items()))
    if key in _BUILT:
        return _BUILT[key]
    import concourse.bacc as bacc
    import concourse.tile as tile
    import concourse.mybir as mybir

    f32 = mybir.dt.float32
    bf16 = mybir.dt.bfloat16
    u32 = mybir.dt.uint32
    AF = mybir.ActivationFunctionType
    OP = mybir.AluOpType
    AX = mybir.AxisListType

    nc = bacc.Bacc("TRN2", target_bir_lowering=False, debug=False, num_devices=8)

    patches_d = nc.dram_tensor("patches", [K, SLOTS, NPIX], bf16, kind="ExternalInput")
    w_d = nc.dram_tensor("w", [K, OUTCH], bf16, kind="ExternalInput")
    bias_d = nc.dram_tensor("bias", [128, OUTCH], f32, kind="ExternalInput")
    ident_d = nc.dram_tensor("ident", [128, 128], f32, kind="ExternalInput")
    identb_d = nc.dram_tensor("identb", [128, 128], bf16, kind="ExternalInput")
    out_d = nc.dram_tensor("out", [2, 128, NPIX], f32, kind="ExternalOutput")

    with tile.TileContext(nc) as tc:
        with (
            tc.tile_pool(name="const", bufs=1) as const,
            tc.tile_pool(name="pat", bufs=3) as patp,
            tc.tile_pool(name="votes", bufs=4) as votesp,
            tc.tile_pool(name="big", bufs=CFG["big_bufs"]) as bigp,
            tc.tile_pool(name="state", bufs=3) as statep,
            tc.tile_pool(name="obuf", bufs=1) as obufp,
            tc.tile_pool(name="pconv", bufs=CFG["pconv_bufs"], space="PSUM") as pconv,
            tc.tile_pool(name="pdist", bufs=2, space="PSUM") as pdist,
            tc.tile_pool(name="ptr", bufs=1, space="PSUM") as ptr,
        ):
            w_sb = const.tile([K, OUTCH], bf16)
            nc.sync.dma_start(w_sb[:], w_d.ap())
            bias_sb = const.tile([128, OUTCH], f32)
            nc.sync.dma_start(bias_sb[:], bias_d.ap())
            ident_sb = const.tile([128, 128], f32)
            nc.sync.dma_start(ident_sb[:], ident_d.ap())
            identb_sb = const.tile([128, 128], bf16)
            nc.sync.dma_start(identb_sb[:], identb_d.ap())
            ones1 = const.tile([1, 128], f32)
            nc.gpsimd.memset(ones1[:], 1.0)
            qc = const.tile([128, CO], u32)
            nc.gpsimd.memset(qc[:], QK)

            ob = [
                obufp.tile([128, NPIX], f32, tag=f"ob{h}", name=f"ob{h}")
                for h in range(2)
            ]

            def conv_tile(t):
                # votes for 128 pixels; Uxs slot first so iteration 1 can
                # start early; ci-pairs share one PSUM bank so the Act
                # evacuation runs half as many, double-width copies.
                pt = patp.tile([K, SLOTS, TP], bf16, tag="pt", name=f"pt{t}")
                nc.sync.dma_start(
                    pt[:, CI, :], patches_d.ap()[:, CI, t * TP : (t + 1) * TP]
                )
                nc.sync.dma_start(
                    pt[:, :CI, :], patches_d.ap()[:, :CI, t * TP : (t + 1) * TP]
                )
                U = votesp.tile([128, CI, NO, CO], bf16, tag="U", name=f"U{t}")
                Uxs = votesp.tile([128, OUTCH], f32, tag="Uxs", name=f"Uxs{t}")
                conv_tile.out[t] = (U, Uxs)
                pvx = pconv.tile([128, 2 * OUTCH], f32, tag="pv", name=f"pvx{t}")
                nc.tensor.matmul(
                    pvx[:, :OUTCH], pt[:, CI, :], w_sb[:], start=True, stop=True
                )
                nc.scalar.copy(Uxs[:], pvx[:, :OUTCH])
                yield
                for c in range(CI // 2):
                    pv = pconv.tile([128, 2 * OUTCH], f32, tag="pv",
                                    name=f"pv{t}_{c}")
                    nc.tensor.matmul(
                        pv[:, :OUTCH], pt[:, 2 * c, :], w_sb[:],
                        start=True, stop=True,
                    )
                    nc.tensor.matmul(
                        pv[:, OUTCH:], pt[:, 2 * c + 1, :], w_sb[:],
                        start=True, stop=True,
                    )
                    dst = U[:, 2 * c : 2 * c + 2].rearrange(
                        "p c n o -> p (c n o)"
                    )
                    nc.scalar.copy(dst, pv[:])
                    yield
            conv_tile.out = {}

            def emit_out(t, V):
                Vf = V[:].rearrange("p n c -> p (n c)")
                for h in range(2):
                    tp = ptr.tile([128, 128], f32, tag="tp", name=f"tp{t}_{h}")
                    nc.tensor.transpose(
                        tp[:], Vf[:, h * 128 : (h + 1) * 128], ident_sb[:]
                    )
                    nc.scalar.copy(ob[h][:, t * TP : (t + 1) * TP], tp[:])
                    nc.sync.dma_start(
                        out_d.ap()[h][:, t * TP : (t + 1) * TP],
                        ob[h][:, t * TP : (t + 1) * TP],
                    )

            def squash(t, S, it, out_dtype, newton):
                # S: [128, NO, CO] f32 (SBUF or PSUM view) -> V [128, NO, CO]
                # scl = sqrt(n)/(1+n) via Quake rsqrt (no act tables needed)
                sq = statep.tile([128, NO, CO], f32, tag="sq", name=f"sq{t}_{it}")
                nc.scalar.square(sq[:], S)
                nsq = statep.tile([128, CO], f32, tag="nsq", name=f"nsq{t}_{it}")
                nc.vector.tensor_reduce(
                    nsq[:], sq[:].transpose([0, 2, 1]), axis=AX.X, op=OP.add
                )
                yield
                sh = statep.tile([128, CO], u32, tag="sh", name=f"sh{t}_{it}")
                nc.vector.tensor_scalar(
                    sh[:], nsq[:].bitcast(u32), 1, None,
                    op0=OP.logical_shift_right,
                )
                y = statep.tile([128, CO], f32, tag="y", name=f"y{t}_{it}")
                nc.vector.tensor_tensor(
                    y[:].bitcast(u32), qc[:], sh[:], op=OP.subtract
                )
                den = statep.tile([128, CO], f32, tag="den", name=f"den{t}_{it}")
                nc.vector.tensor_scalar_add(den[:], nsq[:], 1.0)
                rcd = statep.tile([128, CO], f32, tag="rcd", name=f"rcd{t}_{it}")
                nc.vector.reciprocal(rcd[:], den[:])
                tq = statep.tile([128, CO], f32, tag="tq", name=f"tq{t}_{it}")
                for _ in range(newton):
                    nc.vector.tensor_mul(tq[:], y[:], y[:])
                    nc.vector.tensor_mul(tq[:], tq[:], nsq[:])
                    nc.vector.tensor_scalar(
                        tq[:], tq[:], -0.5, 1.5, op0=OP.mult, op1=OP.add
                    )
                    nc.vector.tensor_mul(y[:], y[:], tq[:])
                yield
                # scl = nsq * y * rcd  (= sqrt(nsq)/(1+nsq))
                scl = statep.tile([128, CO], f32, tag="scl", name=f"scl{t}_{it}")
                nc.vector.tensor_mul(scl[:], nsq[:], y[:])
                nc.vector.tensor_mul(scl[:], scl[:], rcd[:])
                V = statep.tile([128, NO, CO], out_dtype, tag=f"V{it}",
                                name=f"V{t}_{it}")
                nc.vector.tensor_mul(
                    V[:], S, scl[:].unsqueeze(1).broadcast_to([128, NO, CO])
                )
                yield
                squash.out = V

            def s_phase(t, U, R, it):
                # Fused: tmp = U*R (bf16 2x, ci-quarters) pipelined into the
                # PE ci-contraction. Identity stays the stationary, so each
                # matmul is a PSUM-accumulating copy; bias opens the group
                # as a rank-1 ones x bias_row matmul. S lands in [p,(no,co)]
                # f32 PSUM exactly.
                SBt = pdist.tile([128, CI * CO], f32, tag="D",
                                 name=f"SB{t}_{it}")
                SB = SBt[:, :OUTCH]
                tmp = bigp.tile([128, CI, NO, CO], bf16, tag="tmp",
                                name=f"tmps{it}_{t}")
                facb = R[:].unsqueeze(2).broadcast_to([128, CI, NO, CO])
                nc.tensor.matmul(
                    SB, ones1[:], bias_sb[0:1, :],
                    start=True, stop=False, skip_group_check=True,
                )
                Q = CI // 2
                for q in range(2):
                    sl = slice(q * Q, (q + 1) * Q)
                    nc.vector.tensor_mul(tmp[:, sl], U[:, sl], facb[:, sl])
                    yield
                    for ci in range(q * Q, (q + 1) * Q):
                        nc.tensor.matmul(
                            SB, identb_sb[:],
                            tmp[:, ci].rearrange("p n c -> p (n c)"),
                            start=False, stop=(ci == CI - 1),
                            skip_group_check=True,
                        )
                    yield
                s_phase.out = SB

            def d_phase(t, U, V, Dps, it):
                # Fused: tmpn = U*V in no-major layout (strided write keeps
                # co innermost -> DVE 2x survives), pipelined in no-halves
                # into PE accumulating copies: D[p,(ci,co)] = sum_no tmpn.
                # The ci-half h=0 finishes first so the softmax can start
                # on it while h=1 accumulates.
                tmpn = bigp.tile([128, NO, CI, CO], bf16, tag="tmp",
                                 name=f"tmpd{it}_{t}")
                tmp = tmpn[:].transpose([0, 2, 1, 3])
                facb = V[:].unsqueeze(1).broadcast_to([128, CI, NO, CO])
                mvs = [tmpn[:, no].rearrange("p c o -> p (c o)")
                       for no in range(NO)]
                HN = NO // 2
                for g in range(2):
                    nsl = slice(g * HN, (g + 1) * HN)
                    nc.vector.tensor_mul(
                        tmp[:, :, nsl], U[:, :, nsl], facb[:, :, nsl]
                    )
                    yield
                    if g == 0:
                        for h in range(2):
                            for no in range(HN):
                                nc.tensor.matmul(
                                    Dps[:, h * 512 : (h + 1) * 512],
                                    identb_sb[:],
                                    mvs[no][:, h * 512 : (h + 1) * 512],
                                    start=(no == 0), stop=False,
                                    skip_group_check=True,
                                )
                        yield
                    else:
                        for h in range(2):
                            for no in range(HN, NO):
                                nc.tensor.matmul(
                                    Dps[:, h * 512 : (h + 1) * 512],
                                    identb_sb[:],
                                    mvs[no][:, h * 512 : (h + 1) * 512],
                                    start=False, stop=(no == NO - 1),
                                    skip_group_check=True,
                                )
                            yield

            def routing_tile(t, U, Uxs):
                # ---- iteration 1: route is uniform 1/CI ----
                S1 = statep.tile([128, NO, CO], f32, tag="S", name=f"S1_{t}")
                nc.vector.scalar_tensor_tensor(
                    S1[:].rearrange("p n c -> p (n c)"), Uxs[:], 1.0 / CI,
                    bias_sb[:], op0=OP.mult, op1=OP.add,
                )
                yield
                yield from squash(t, S1[:], 1, bf16, CFG["newton12"])
                V1 = squash.out
                Dps = pdist.tile([128, CI * CO], f32, tag="D", name=f"D{t}_1")
                yield from d_phase(t, U, V1, Dps, 1)
                # ---- iterations 2, 3 ----
                V = None
                Eprev = None
                HC = CI // 2
                for it in (2, 3):
                    # softmax over co in ci-halves; logits stay in PSUM
                    # (exact f32). Iteration 3 uses exp(D1+D2) =
                    # exp(D1)*exp(D2) so the distance tile only lives
                    # product -> exp.
                    E = statep.tile([128, CI, CO], bf16, tag="E",
                                    name=f"E{t}_{it}")
                    R = statep.tile([128, CI, CO], bf16, tag="R",
                                    name=f"R{t}_{it}")
                    sume = statep.tile([128, CI], f32, tag="sume",
                                       name=f"sume{t}_{it}")
                    rec = statep.tile([128, CI], f32, tag="rec",
                                      name=f"rec{t}_{it}")
                    for h in range(2):
                        hs = slice(h * HC, (h + 1) * HC)
                        Lv = Dps[:, h * 512 : (h + 1) * 512].rearrange(
                            "p (i c) -> p i c", i=HC
                        )
                        nc.scalar.activation(E[:, hs], Lv, AF.Exp)
                        if it == 3:
                            nc.vector.tensor_mul(
                                E[:, hs], E[:, hs], Eprev[:, hs]
                            )
                        nc.vector.tensor_reduce(
                            sume[:, hs], E[:, hs], axis=AX.X, op=OP.add
                        )
                        nc.vector.reciprocal(rec[:, hs], sume[:, hs])
                        nc.vector.tensor_mul(
                            R[:, hs], E[:, hs],
                            rec[:, hs].unsqueeze(2).broadcast_to(
                                [128, HC, CO]
                            ),
                        )
                        yield
                    Eprev = E
                    yield from s_phase(t, U, R, it)
                    SB = s_phase.out
                    Sv = SB.rearrange("p (n c) -> p n c", n=NO)
                    if it == 2:
                        yield from squash(t, Sv, it, bf16, CFG["newton12"])
                        V = squash.out
                        Dps = pdist.tile([128, CI * CO], f32, tag="D",
                                         name=f"D{t}_2")
                        yield from d_phase(t, U, V, Dps, 2)
                    else:
                        yield from squash(t, Sv, it, f32, CFG["newton3"])
                        V = squash.out
                emit_out(t, V)

            def drain(gens):
                alive = [g for g in gens if g is not None]
                while alive:
                    for g in list(alive):
                        try:
                            next(g)
                        except StopIteration:
                            alive.remove(g)

            if CFG["pair"]:
                groups = [(0, 1), (2, 3), (4, 5), (6, 7)]
                drain([conv_tile(t) for t in groups[0]])
                for gi, grp in enumerate(groups):
                    gens = [routing_tile(t, *conv_tile.out[t]) for t in grp]
                    if gi + 1 < len(groups):
                        gens += [conv_tile(t) for t in groups[gi + 1]]
                    drain(gens)
            else:
                for t in range(TILES):
                    drain([conv_tile(t)])
                    drain([routing_tile(t, *conv_tile.out[t])])

    nc.compile()
    _BUILT[key] = nc
    return nc


def _assemble(out_halves_all):
    o = out_halves_all.reshape(-1, 2, 4, CO, NPIX)
    return np.ascontiguousarray(
        o.transpose(0, 3, 1, 2, 4).reshape(-1, CO, NO, H, W)
    )


def kernel(x, conv_w, bias):
    import sys
    if "/opt/trn_rl_repo" not in sys.path:
        sys.path.insert(0, "/opt/trn_rl_repo")
    from concourse import bass_utils

    patches, w_m, bias_bc, ident, identb = _host_prep(x, conv_w, bias)
    nc = _build_nc()
    in_maps = [
        {"patches": patches[b], "w": w_m, "bias": bias_bc, "ident": ident,
         "identb": identb}
        for b in range(BS)
    ]
    res = bass_utils.run_bass_kernel_spmd(nc, in_maps, core_ids=list(range(BS)))
    outs = np.stack([r["out"] for r in res.results])
    return _assemble(outs).astype(np.float32)


# revision 18
# speedup vs baseline: 1.0209x; 1.0029x over previous
"""Trainium2 Bass kernel for ConvPixelToCapsules (conv -> 3-iter dynamic routing).

Strategy (hardcoded for x[8,32,8,32,32], conv_w[256,8,3,3], bias[32,8,1,1]):
  - Host precomputes im2col patches per batch element, with an extra 33rd
    "channel" slot holding sum_ci(x) (conv linearity gives iteration-1's
    uniform-route preactivation for free), plus the weight matrix in
    [72, (no,co)] layout and a partition-broadcast bias tile.
  - 8 NeuronCores, data-parallel over batch: core k owns batch element k.
  - Per core: 8 tiles of 128 output pixels. Votes live in SBUF as
    [pixel-partition; (ci,no,co)] bf16; all products are DVE bf16 2x ops.
  - v3: the ci-contraction (preactivation S = sum_ci R*U) runs on the PE as
    32 accumulating identity-matmuls (exact f32 PSUM accumulation, bias
    folded in as a rank-1 matmul), transposed back by the PE. The no-
    contraction (distances) stays a DVE halving tree with the final level
    on GPSIMD. Squash computes sqrt via Quake-rsqrt + Newton on DVE
    (bitcast/shift ops), so the Act engine only ever needs Copy+Exp: one
    activation-table load for the whole program instead of 43.
  - PSUM conv evacuation is paired ([128,512] per copy) to halve Act time.
"""

import numpy as np

BS, CI, NI, H, W = 8, 32, 8, 32, 32
CO, NO = 32, 8
NPIX = H * W            # 1024
TILES = 8               # tiles of 128 pixels per batch element
TP = 128                # pixels per tile (on partitions)
K = 72                  # ni * 3 * 3 contraction
SLOTS = CI + 1          # 32 ci + xsum slot
OUTCH = NO * CO         # 256, (no, co) order
QK = 0x5F3759DF         # Quake rsqrt seed constant

CFG = {
    "pair": True,          # interleave emission of tile pairs
    "newton12": 1,         # Newton iterations for squash 1-2
    "newton3": 2,          # Newton iterations for final squash
    "bias_mm": True,       # fold bias add into the PE accumulation group
    "big_bufs": 3,
    "pconv_bufs": 2,
}

_BUILT = {}


def _host_prep(x, conv_w, bias):
    x = np.asarray(x, np.float32)
    conv_w = np.asarray(conv_w, np.float32)
    bias = np.asarray(bias, np.float32)
    x_pad = np.pad(x, ((0, 0), (0, 0), (0, 0), (1, 1), (1, 1)))
    x_aug = np.concatenate([x_pad, x_pad.sum(1, keepdims=True)], axis=1)
    wv = np.lib.stride_tricks.sliding_window_view(x_aug, (3, 3), axis=(3, 4))
    import ml_dtypes
    cdt_np = ml_dtypes.bfloat16
    patches = np.ascontiguousarray(
        wv.transpose(0, 2, 5, 6, 1, 3, 4).reshape(BS, K, SLOTS, NPIX)
    ).astype(cdt_np)
    w_m = np.ascontiguousarray(
        conv_w.reshape(CO, NO, NI, 3, 3).transpose(2, 3, 4, 1, 0).reshape(K, OUTCH)
    ).astype(cdt_np)
    bias_bc = np.broadcast_to(
        bias[:, :, 0, 0].T.reshape(1, OUTCH), (128, OUTCH)
    ).astype(np.float32)
    ident = np.eye(128, dtype=np.float32)
    identb = np.eye(128, dtype=cdt_np)
    return patches, w_m, bias_bc, ident, identb


def _build_nc():
    key = ("nc",) + tuple(sorted(CFG.items()))
    if key in _BUILT:
        return _BUILT[key]
    import concourse.bacc as bacc
    import concourse.tile as tile
    import concourse.mybir as mybir

    f32 = mybir.dt.float32
    bf16 = mybir.dt.bfloat16
    u32 = mybir.dt.uint32
    AF = mybir.ActivationFunctionType
    OP = mybir.AluOpType
    AX = mybir.AxisListType

    nc = bacc.Bacc("TRN2", target_bir_lowering=False, debug=False, num_devices=8)

    patches_d = nc.dram_tensor("patches", [K, SLOTS, NPIX], bf16, kind="ExternalInput")
    w_d = nc.dram_tensor("w", [K, OUTCH], bf16, kind="ExternalInput")
    bias_d = nc.dram_tensor("bias", [128, OUTCH], f32, kind="ExternalInput")
    ident_d = nc.dram_tensor("ident", [128, 128], f32, kind="ExternalInput")
    identb_d = nc.dram_tensor("identb", [128, 128], bf16, kind="ExternalInput")
    out_d = nc.dram_tensor("out", [2, 128, NPIX], f32, kind="ExternalOutput")

    with tile.TileContext(nc) as tc:
        with (
            tc.tile_pool(name="const", bufs=1) as const,
            tc.tile_pool(name="pat", bufs=3) as patp,
            tc.tile_pool(name="votes", bufs=4) as votesp,
            tc.tile_pool(name="big", bufs=CFG["big_bufs"]) as bigp,
            tc.tile_pool(name="state", bufs=3) as statep,
            tc.tile_pool(name="obuf", bufs=1) as obufp,
            tc.tile_pool(name="pconv", bufs=CFG["pconv_bufs"], space="PSUM") as pconv,
            tc.tile_pool(name="pdist", bufs=2, space="PSUM") as pdist,
            tc.tile_pool(name="ptr", bufs=1, space="PSUM") as ptr,
        ):
            w_sb = const.tile([K, OUTCH], bf16)
            nc.sync.dma_start(w_sb[:], w_d.ap())
            bias_sb = const.tile([128, OUTCH], f32)
            nc.sync.dma_start(bias_sb[:], bias_d.ap())
            ident_sb = const.tile([128, 128], f32)
            nc.sync.dma_start(ident_sb[:], ident_d.ap())
            identb_sb = const.tile([128, 128], bf16)
            nc.sync.dma_start(identb_sb[:], identb_d.ap())
            ones1 = const.tile([1, 128], f32)
            nc.gpsimd.memset(ones1[:], 1.0)
            qc = const.tile([128, CO], u32)
            nc.gpsimd.memset(qc[:], QK)

            ob = [
                obufp.tile([128, NPIX], f32, tag=f"ob{h}", name=f"ob{h}")
                for h in range(2)
            ]

            def conv_tile(t):
                # votes for 128 pixels; Uxs slot first so iteration 1 can
                # start early; ci-pairs share one PSUM bank so the Act
                # evacuation runs half as many, double-width copies.
                pt = patp.tile([K, SLOTS, TP], bf16, tag="pt", name=f"pt{t}")
                nc.sync.dma_start(
                    pt[:, CI, :], patches_d.ap()[:, CI, t * TP : (t + 1) * TP]
                )
                nc.sync.dma_start(
                    pt[:, :CI, :], patches_d.ap()[:, :CI, t * TP : (t + 1) * TP]
                )
                U = votesp.tile([128, CI, NO, CO], bf16, tag="U", name=f"U{t}")
                Uxs = votesp.tile([128, OUTCH], f32, tag="Uxs", name=f"Uxs{t}")
                conv_tile.out[t] = (U, Uxs)
                pvx = pconv.tile([128, 2 * OUTCH], f32, tag="pv", name=f"pvx{t}")
                nc.tensor.matmul(
                    pvx[:, :OUTCH], pt[:, CI, :], w_sb[:], start=True, stop=True
                )
                nc.scalar.copy(Uxs[:], pvx[:, :OUTCH])
                yield
                for c in range(CI // 2):
                    pv = pconv.tile([128, 2 * OUTCH], f32, tag="pv",
                                    name=f"pv{t}_{c}")
                    nc.tensor.matmul(
                        pv[:, :OUTCH], pt[:, 2 * c, :], w_sb[:],
                        start=True, stop=True,
                    )
                    nc.tensor.matmul(
                        pv[:, OUTCH:], pt[:, 2 * c + 1, :], w_sb[:],
                        start=True, stop=True,
                    )
                    dst = U[:, 2 * c : 2 * c + 2].rearrange(
                        "p c n o -> p (c n o)"
                    )
                    nc.scalar.copy(dst, pv[:])
                    yield
            conv_tile.out = {}

            def emit_out(t, V):
                Vf = V[:].rearrange("p n c -> p (n c)")
                for h in range(2):
                    tp = ptr.tile([128, 128], f32, tag="tp", name=f"tp{t}_{h}")
                    nc.tensor.transpose(
                        tp[:], Vf[:, h * 128 : (h + 1) * 128], ident_sb[:]
                    )
                    nc.scalar.copy(ob[h][:, t * TP : (t + 1) * TP], tp[:])
                    nc.sync.dma_start(
                        out_d.ap()[h][:, t * TP : (t + 1) * TP],
                        ob[h][:, t * TP : (t + 1) * TP],
                    )

            def squash(t, S, it, out_dtype, newton):
                # S: [128, NO, CO] f32 (SBUF or PSUM view) -> V [128, NO, CO]
                # scl = sqrt(n)/(1+n) via Quake rsqrt (no act tables needed)
                sq = statep.tile([128, NO, CO], f32, tag="sq", name=f"sq{t}_{it}")
                nc.scalar.square(sq[:], S)
                nsq = statep.tile([128, CO], f32, tag="nsq", name=f"nsq{t}_{it}")
                nc.vector.tensor_reduce(
                    nsq[:], sq[:].transpose([0, 2, 1]), axis=AX.X, op=OP.add
                )
                yield
                sh = statep.tile([128, CO], u32, tag="sh", name=f"sh{t}_{it}")
                nc.vector.tensor_scalar(
                    sh[:], nsq[:].bitcast(u32), 1, None,
                    op0=OP.logical_shift_right,
                )
                y = statep.tile([128, CO], f32, tag="y", name=f"y{t}_{it}")
                nc.vector.tensor_tensor(
                    y[:].bitcast(u32), qc[:], sh[:], op=OP.subtract
                )
                den = statep.tile([128, CO], f32, tag="den", name=f"den{t}_{it}")
                nc.vector.tensor_scalar_add(den[:], nsq[:], 1.0)
                rcd = statep.tile([128, CO], f32, tag="rcd", name=f"rcd{t}_{it}")
                nc.vector.reciprocal(rcd[:], den[:])
                tq = statep.tile([128, CO], f32, tag="tq", name=f"tq{t}_{it}")
                for _ in range(newton):
                    nc.vector.tensor_mul(tq[:], y[:], y[:])
                    nc.vector.tensor_mul(tq[:], tq[:], nsq[:])
                    nc.vector.tensor_scalar(
                        tq[:], tq[:], -0.5, 1.5, op0=OP.mult, op1=OP.add
                    )
                    nc.vector.tensor_mul(y[:], y[:], tq[:])
                yield
                # scl = nsq * y * rcd  (= sqrt(nsq)/(1+nsq))
                scl = statep.tile([128, CO], f32, tag="scl", name=f"scl{t}_{it}")
                nc.vector.tensor_mul(scl[:], nsq[:], y[:])
                nc.vector.tensor_mul(scl[:], scl[:], rcd[:])
                V = statep.tile([128, NO, CO], out_dtype, tag=f"V{it}",
                                name=f"V{t}_{it}")
                nc.vector.tensor_mul(
                    V[:], S, scl[:].unsqueeze(1).broadcast_to([128, NO, CO])
                )
                yield
                squash.out = V

            def s_phase(t, U, R, it):
                # Fused: tmp = U*R (bf16 2x, ci-quarters) pipelined into the
                # PE ci-contraction. Identity stays the stationary, so each
                # matmul is a PSUM-accumulating copy; bias opens the group
                # as a rank-1 ones x bias_row matmul. S lands in [p,(no,co)]
                # f32 PSUM exactly.
                SBt = pdist.tile([128, CI * CO], f32, tag="D",
                                 name=f"SB{t}_{it}")
                SB = SBt[:, :OUTCH]
                tmp = bigp.tile([128, CI, NO, CO], bf16, tag="tmp",
                                name=f"tmps{it}_{t}")
                facb = R[:].unsqueeze(2).broadcast_to([128, CI, NO, CO])
                nc.tensor.matmul(
                    SB, ones1[:], bias_sb[0:1, :],
                    start=True, stop=False, skip_group_check=True,
                )
                Q = CI // 2
                for q in range(2):
                    sl = slice(q * Q, (q + 1) * Q)
                    nc.vector.tensor_mul(tmp[:, sl], U[:, sl], facb[:, sl])
                    yield
                    for ci in range(q * Q, (q + 1) * Q):
                        nc.tensor.matmul(
                            SB, identb_sb[:],
                            tmp[:, ci].rearrange("p n c -> p (n c)"),
                            start=False, stop=(ci == CI - 1),
                            skip_group_check=True,
                        )
                    yield
                s_phase.out = SB

            def d_phase(t, U, V, Dps, it):
                # Fused: tmpn = U*V in no-major layout (strided write keeps
                # co innermost -> DVE 2x survives), pipelined in no-halves
                # into PE accumulating copies: D[p,(ci,co)] = sum_no tmpn.
                # The ci-half h=0 finishes first so the softmax can start
                # on it while h=1 accumulates.
                tmpn = bigp.tile([128, NO, CI, CO], bf16, tag="tmp",
                                 name=f"tmpd{it}_{t}")
                tmp = tmpn[:].transpose([0, 2, 1, 3])
                facb = V[:].unsqueeze(1).broadcast_to([128, CI, NO, CO])
                mvs = [tmpn[:, no].rearrange("p c o -> p (c o)")
                       for no in range(NO)]
                HN = NO // 2
                for g in range(2):
                    nsl = slice(g * HN, (g + 1) * HN)
                    nc.vector.tensor_mul(
                        tmp[:, :, nsl], U[:, :, nsl], facb[:, :, nsl]
                    )
                    yield
                    if g == 0:
                        for h in range(2):
                            for no in range(HN):
                                nc.tensor.matmul(
                                    Dps[:, h * 512 : (h + 1) * 512],
                                    identb_sb[:],
                                    mvs[no][:, h * 512 : (h + 1) * 512],
                                    start=(no == 0), stop=False,
                                    skip_group_check=True,
                                )
                        yield
                    else:
                        for h in range(2):
                            for no in range(HN, NO):
                                nc.tensor.matmul(
                                    Dps[:, h * 512 : (h + 1) * 512],
                                    identb_sb[:],
                                    mvs[no][:, h * 512 : (h + 1) * 512],
                                    start=False, stop=(no == NO - 1),
                                    skip_group_check=True,
                                )
                            yield

            def routing_tile(t, U, Uxs):
                # ---- iteration 1: route is uniform 1/CI ----
                S1 = statep.tile([128, NO, CO], f32, tag="S", name=f"S1_{t}")
                nc.vector.scalar_tensor_tensor(
                    S1[:].rearrange("p n c -> p (n c)"), Uxs[:], 1.0 / CI,
                    bias_sb[:], op0=OP.mult, op1=OP.add,
                )
                yield
                yield from squash(t, S1[:], 1, bf16, CFG["newton12"])
                V1 = squash.out
                Dps = pdist.tile([128, CI * CO], f32, tag="D", name=f"D{t}_1")
                yield from d_phase(t, U, V1, Dps, 1)
                # ---- iterations 2, 3 ----
                V = None
                Eprev = None
                HC = CI // 2
                for it in (2, 3):
                    # softmax over co in ci-halves; logits stay in PSUM
                    # (exact f32). Iteration 3 uses exp(D1+D2) =
                    # exp(D1)*exp(D2) so the distance tile only lives
                    # product -> exp.
                    E = statep.tile([128, CI, CO], bf16, tag="E",
                                    name=f"E{t}_{it}")
                    R = statep.tile([128, CI, CO], bf16, tag="R",
                                    name=f"R{t}_{it}")
                    sume = statep.tile([128, CI], f32, tag="sume",
                                       name=f"sume{t}_{it}")
                    rec = statep.tile([128, CI], f32, tag="rec",
                                      name=f"rec{t}_{it}")
                    Lv = Dps[:].rearrange("p (i c) -> p i c", i=CI)
                    nc.scalar.activation(E[:], Lv, AF.Exp)
                    if it == 3:
                        nc.vector.tensor_mul(E[:], E[:], Eprev[:])
                    nc.vector.tensor_reduce(
                        sume[:], E[:], axis=AX.X, op=OP.add
                    )
                    nc.vector.reciprocal(rec[:], sume[:])
                    yield
                    nc.vector.tensor_mul(
                        R[:], E[:],
                        rec[:].unsqueeze(2).broadcast_to([128, CI, CO]),
                    )
                    yield
                    Eprev = E
                    yield from s_phase(t, U, R, it)
                    SB = s_phase.out
                    Sv = SB.rearrange("p (n c) -> p n c", n=NO)
                    if it == 2:
                        yield from squash(t, Sv, it, bf16, CFG["newton12"])
                        V = squash.out
                        Dps = pdist.tile([128, CI * CO], f32, tag="D",
                                         name=f"D{t}_2")
                        yield from d_phase(t, U, V, Dps, 2)
                    else:
                        yield from squash(t, Sv, it, f32, CFG["newton3"])
                        V = squash.out
                emit_out(t, V)

            def drain(gens):
                alive = [g for g in gens if g is not None]
                while alive:
                    for g in list(alive):
                        try:
                            next(g)
                        except StopIteration:
                            alive.remove(g)

            if CFG["pair"]:
                groups = [(0, 1), (2, 3), (4, 5), (6, 7)]
                drain([conv_tile(t) for t in groups[0]])
                for gi, grp in enumerate(groups):
                    gens = [routing_tile(t, *conv_tile.out[t]) for t in grp]
                    if gi + 1 < len(groups):
                        gens += [conv_tile(t) for t in groups[gi + 1]]
                    drain(gens)
            else:
                for t in range(TILES):
                    drain([conv_tile(t)])
                    drain([routing_tile(t, *conv_tile.out[t])])

    nc.compile()
    _BUILT[key] = nc
    return nc


def _assemble(out_halves_all):
    o = out_halves_all.reshape(-1, 2, 4, CO, NPIX)
    return np.ascontiguousarray(
        o.transpose(0, 3, 1, 2, 4).reshape(-1, CO, NO, H, W)
    )


def kernel(x, conv_w, bias):
    import sys
    if "/opt/trn_rl_repo" not in sys.path:
        sys.path.insert(0, "/opt/trn_rl_repo")
    from concourse import bass_utils

    patches, w_m, bias_bc, ident, identb = _host_prep(x, conv_w, bias)
    nc = _build_nc()
    in_maps = [
        {"patches": patches[b], "w": w_m, "bias": bias_bc, "ident": ident,
         "identb": identb}
        for b in range(BS)
    ]
    res = bass_utils.run_bass_kernel_spmd(nc, in_maps, core_ids=list(range(BS)))
    outs = np.stack([r["out"] for r in res.results])
    return _assemble(outs).astype(np.float32)


# revision 19
# speedup vs baseline: 1.0311x; 1.0100x over previous
"""Trainium2 Bass kernel for ConvPixelToCapsules (conv -> 3-iter dynamic routing).

Strategy (hardcoded for x[8,32,8,32,32], conv_w[256,8,3,3], bias[32,8,1,1]):
  - Host precomputes im2col patches per batch element, with an extra 33rd
    "channel" slot holding sum_ci(x) (conv linearity gives iteration-1's
    uniform-route preactivation for free), plus the weight matrix in
    [72, (no,co)] layout and a partition-broadcast bias tile.
  - 8 NeuronCores, data-parallel over batch: core k owns batch element k.
  - Per core: 8 tiles of 128 output pixels. Votes live in SBUF as
    [pixel-partition; (ci,no,co)] bf16; all products are DVE bf16 2x ops.
  - v3: the ci-contraction (preactivation S = sum_ci R*U) runs on the PE as
    32 accumulating identity-matmuls (exact f32 PSUM accumulation, bias
    folded in as a rank-1 matmul), transposed back by the PE. The no-
    contraction (distances) stays a DVE halving tree with the final level
    on GPSIMD. Squash computes sqrt via Quake-rsqrt + Newton on DVE
    (bitcast/shift ops), so the Act engine only ever needs Copy+Exp: one
    activation-table load for the whole program instead of 43.
  - PSUM conv evacuation is paired ([128,512] per copy) to halve Act time.
"""

import numpy as np

BS, CI, NI, H, W = 8, 32, 8, 32, 32
CO, NO = 32, 8
NPIX = H * W            # 1024
TILES = 8               # tiles of 128 pixels per batch element
TP = 128                # pixels per tile (on partitions)
K = 72                  # ni * 3 * 3 contraction
SLOTS = CI + 1          # 32 ci + xsum slot
OUTCH = NO * CO         # 256, (no, co) order
QK = 0x5F3759DF         # Quake rsqrt seed constant

CFG = {
    "pair": True,          # interleave emission of tile pairs
    "newton12": 1,         # Newton iterations for squash 1-2
    "newton3": 2,          # Newton iterations for final squash
    "bias_mm": True,       # fold bias add into the PE accumulation group
    "big_bufs": 3,
    "pconv_bufs": 2,
}

_BUILT = {}


def _host_prep(x, conv_w, bias):
    x = np.asarray(x, np.float32)
    conv_w = np.asarray(conv_w, np.float32)
    bias = np.asarray(bias, np.float32)
    x_pad = np.pad(x, ((0, 0), (0, 0), (0, 0), (1, 1), (1, 1)))
    x_aug = np.concatenate([x_pad, x_pad.sum(1, keepdims=True)], axis=1)
    wv = np.lib.stride_tricks.sliding_window_view(x_aug, (3, 3), axis=(3, 4))
    import ml_dtypes
    cdt_np = ml_dtypes.bfloat16
    patches = np.ascontiguousarray(
        wv.transpose(0, 2, 5, 6, 1, 3, 4).reshape(BS, K, SLOTS, NPIX)
    ).astype(cdt_np)
    w_m = np.ascontiguousarray(
        conv_w.reshape(CO, NO, NI, 3, 3).transpose(2, 3, 4, 1, 0).reshape(K, OUTCH)
    ).astype(cdt_np)
    bias_bc = np.broadcast_to(
        bias[:, :, 0, 0].T.reshape(1, OUTCH), (128, OUTCH)
    ).astype(np.float32)
    ident = np.eye(128, dtype=np.float32)
    identb = np.eye(128, dtype=cdt_np)
    return patches, w_m, bias_bc, ident, identb


def _build_nc():
    key = ("nc",) + tuple(sorted(CFG.items()))
    if key in _BUILT:
        return _BUILT[key]
    import concourse.bacc as bacc
    import concourse.tile as tile
    import concourse.mybir as mybir

    f32 = mybir.dt.float32
    bf16 = mybir.dt.bfloat16
    u32 = mybir.dt.uint32
    AF = mybir.ActivationFunctionType
    OP = mybir.AluOpType
    AX = mybir.AxisListType

    nc = bacc.Bacc("TRN2", target_bir_lowering=False, debug=False, num_devices=8)

    patches_d = nc.dram_tensor("patches", [K, SLOTS, NPIX], bf16, kind="ExternalInput")
    w_d = nc.dram_tensor("w", [K, OUTCH], bf16, kind="ExternalInput")
    bias_d = nc.dram_tensor("bias", [128, OUTCH], f32, kind="ExternalInput")
    ident_d = nc.dram_tensor("ident", [128, 128], f32, kind="ExternalInput")
    identb_d = nc.dram_tensor("identb", [128, 128], bf16, kind="ExternalInput")
    out_d = nc.dram_tensor("out", [2, 128, NPIX], f32, kind="ExternalOutput")

    with tile.TileContext(nc) as tc:
        with (
            tc.tile_pool(name="const", bufs=1) as const,
            tc.tile_pool(name="pat", bufs=3) as patp,
            tc.tile_pool(name="votes", bufs=4) as votesp,
            tc.tile_pool(name="big", bufs=CFG["big_bufs"]) as bigp,
            tc.tile_pool(name="state", bufs=3) as statep,
            tc.tile_pool(name="obuf", bufs=1) as obufp,
            tc.tile_pool(name="pconv", bufs=CFG["pconv_bufs"], space="PSUM") as pconv,
            tc.tile_pool(name="pdist", bufs=2, space="PSUM") as pdist,
            tc.tile_pool(name="ptr", bufs=1, space="PSUM") as ptr,
        ):
            w_sb = const.tile([K, OUTCH], bf16)
            nc.sync.dma_start(w_sb[:], w_d.ap())
            bias_sb = const.tile([128, OUTCH], f32)
            nc.sync.dma_start(bias_sb[:], bias_d.ap())
            ident_sb = const.tile([128, 128], f32)
            nc.sync.dma_start(ident_sb[:], ident_d.ap())
            identb_sb = const.tile([128, 128], bf16)
            nc.sync.dma_start(identb_sb[:], identb_d.ap())
            ones1 = const.tile([1, 128], f32)
            nc.gpsimd.memset(ones1[:], 1.0)
            qc = const.tile([128, CO], u32)
            nc.gpsimd.memset(qc[:], QK)

            ob = [
                obufp.tile([128, NPIX], f32, tag=f"ob{h}", name=f"ob{h}")
                for h in range(2)
            ]

            def conv_tile(t):
                # votes for 128 pixels; Uxs slot first so iteration 1 can
                # start early; ci-pairs share one PSUM bank so the Act
                # evacuation runs half as many, double-width copies.
                pt = patp.tile([K, SLOTS, TP], bf16, tag="pt", name=f"pt{t}")
                nc.sync.dma_start(
                    pt[:, CI, :], patches_d.ap()[:, CI, t * TP : (t + 1) * TP]
                )
                nc.sync.dma_start(
                    pt[:, :CI, :], patches_d.ap()[:, :CI, t * TP : (t + 1) * TP]
                )
                U = votesp.tile([128, CI, NO, CO], bf16, tag="U", name=f"U{t}")
                Uxs = votesp.tile([128, OUTCH], f32, tag="Uxs", name=f"Uxs{t}")
                conv_tile.out[t] = (U, Uxs)
                pvx = pconv.tile([128, 2 * OUTCH], f32, tag="pv", name=f"pvx{t}")
                nc.tensor.matmul(
                    pvx[:, :OUTCH], pt[:, CI, :], w_sb[:], start=True, stop=True
                )
                nc.scalar.copy(Uxs[:], pvx[:, :OUTCH])
                yield
                for c in range(CI // 2):
                    pv = pconv.tile([128, 2 * OUTCH], f32, tag="pv",
                                    name=f"pv{t}_{c}")
                    nc.tensor.matmul(
                        pv[:, :OUTCH], pt[:, 2 * c, :], w_sb[:],
                        start=True, stop=True,
                    )
                    nc.tensor.matmul(
                        pv[:, OUTCH:], pt[:, 2 * c + 1, :], w_sb[:],
                        start=True, stop=True,
                    )
                    dst = U[:, 2 * c : 2 * c + 2].rearrange(
                        "p c n o -> p (c n o)"
                    )
                    nc.scalar.copy(dst, pv[:])
                    yield
            conv_tile.out = {}

            def emit_out(t, V):
                Vf = V[:].rearrange("p n c -> p (n c)")
                for h in range(2):
                    tp = ptr.tile([128, 128], f32, tag="tp", name=f"tp{t}_{h}")
                    nc.tensor.transpose(
                        tp[:], Vf[:, h * 128 : (h + 1) * 128], ident_sb[:]
                    )
                    nc.scalar.copy(ob[h][:, t * TP : (t + 1) * TP], tp[:])
                    nc.sync.dma_start(
                        out_d.ap()[h][:, t * TP : (t + 1) * TP],
                        ob[h][:, t * TP : (t + 1) * TP],
                    )

            def squash(t, S, it, out_dtype, newton):
                # S: [128, NO, CO] f32 (SBUF or PSUM view) -> V [128, NO, CO]
                # scl = sqrt(n)/(1+n) via Quake rsqrt (no act tables needed)
                sq = statep.tile([128, NO, CO], f32, tag="sq", name=f"sq{t}_{it}")
                nc.scalar.square(sq[:], S)
                nsq = statep.tile([128, CO], f32, tag="nsq", name=f"nsq{t}_{it}")
                nc.vector.tensor_reduce(
                    nsq[:], sq[:].transpose([0, 2, 1]), axis=AX.X, op=OP.add
                )
                yield
                sh = statep.tile([128, CO], u32, tag="sh", name=f"sh{t}_{it}")
                nc.vector.tensor_scalar(
                    sh[:], nsq[:].bitcast(u32), 1, None,
                    op0=OP.logical_shift_right,
                )
                y = statep.tile([128, CO], f32, tag="y", name=f"y{t}_{it}")
                nc.vector.tensor_tensor(
                    y[:].bitcast(u32), qc[:], sh[:], op=OP.subtract
                )
                den = statep.tile([128, CO], f32, tag="den", name=f"den{t}_{it}")
                nc.vector.tensor_scalar_add(den[:], nsq[:], 1.0)
                rcd = statep.tile([128, CO], f32, tag="rcd", name=f"rcd{t}_{it}")
                nc.vector.reciprocal(rcd[:], den[:])
                tq = statep.tile([128, CO], f32, tag="tq", name=f"tq{t}_{it}")
                for _ in range(newton):
                    nc.vector.tensor_mul(tq[:], y[:], y[:])
                    nc.vector.tensor_mul(tq[:], tq[:], nsq[:])
                    nc.vector.tensor_scalar(
                        tq[:], tq[:], -0.5, 1.5, op0=OP.mult, op1=OP.add
                    )
                    nc.vector.tensor_mul(y[:], y[:], tq[:])
                yield
                # scl = nsq * y * rcd  (= sqrt(nsq)/(1+nsq))
                scl = statep.tile([128, CO], f32, tag="scl", name=f"scl{t}_{it}")
                nc.vector.tensor_mul(scl[:], nsq[:], y[:])
                nc.vector.tensor_mul(scl[:], scl[:], rcd[:])
                V = statep.tile([128, NO, CO], out_dtype, tag=f"V{it}",
                                name=f"V{t}_{it}")
                nc.vector.tensor_mul(
                    V[:], S, scl[:].unsqueeze(1).broadcast_to([128, NO, CO])
                )
                yield
                squash.out = V

            def s_phase(t, U, R, it):
                # Fused: tmp = U*R (bf16 2x, ci-quarters) pipelined into the
                # PE ci-contraction. Identity stays the stationary, so each
                # matmul is a PSUM-accumulating copy; bias opens the group
                # as a rank-1 ones x bias_row matmul. S lands in [p,(no,co)]
                # f32 PSUM exactly.
                SBt = pdist.tile([128, CI * CO], f32, tag="D",
                                 name=f"SB{t}_{it}")
                SB = SBt[:, :OUTCH]
                tmp = bigp.tile([128, CI, NO, CO], bf16, tag="tmp",
                                name=f"tmps{it}_{t}")
                facb = R[:].unsqueeze(2).broadcast_to([128, CI, NO, CO])
                nc.tensor.matmul(
                    SB, ones1[:], bias_sb[0:1, :],
                    start=True, stop=False, skip_group_check=True,
                )
                Q = CI // 2
                for q in range(2):
                    sl = slice(q * Q, (q + 1) * Q)
                    nc.vector.tensor_mul(tmp[:, sl], U[:, sl], facb[:, sl])
                    yield
                    for ci in range(q * Q, (q + 1) * Q):
                        nc.tensor.matmul(
                            SB, identb_sb[:],
                            tmp[:, ci].rearrange("p n c -> p (n c)"),
                            start=False, stop=(ci == CI - 1),
                            skip_group_check=True,
                        )
                    yield
                s_phase.out = SB

            def d_phase(t, U, V, Dps, it):
                # Fused: tmpn = U*V in no-major layout (strided write keeps
                # co innermost -> DVE 2x survives), pipelined in no-halves
                # into PE accumulating copies: D[p,(ci,co)] = sum_no tmpn.
                # The ci-half h=0 finishes first so the softmax can start
                # on it while h=1 accumulates.
                tmpn = bigp.tile([128, NO, CI, CO], bf16, tag="tmp",
                                 name=f"tmpd{it}_{t}")
                tmp = tmpn[:].transpose([0, 2, 1, 3])
                facb = V[:].unsqueeze(1).broadcast_to([128, CI, NO, CO])
                mvs = [tmpn[:, no].rearrange("p c o -> p (c o)")
                       for no in range(NO)]
                HN = NO // 2
                for g in range(2):
                    nsl = slice(g * HN, (g + 1) * HN)
                    nc.vector.tensor_mul(
                        tmp[:, :, nsl], U[:, :, nsl], facb[:, :, nsl]
                    )
                    yield
                    for no in range(g * HN, (g + 1) * HN):
                        for h in range(2):
                            nc.tensor.matmul(
                                Dps[:, h * 512 : (h + 1) * 512],
                                identb_sb[:],
                                mvs[no][:, h * 512 : (h + 1) * 512],
                                start=(no == 0), stop=(no == NO - 1),
                                skip_group_check=True,
                            )
                        if no % 2 == 1:
                            yield

            def routing_tile(t, U, Uxs):
                # ---- iteration 1: route is uniform 1/CI ----
                S1 = statep.tile([128, NO, CO], f32, tag="S", name=f"S1_{t}")
                nc.vector.scalar_tensor_tensor(
                    S1[:].rearrange("p n c -> p (n c)"), Uxs[:], 1.0 / CI,
                    bias_sb[:], op0=OP.mult, op1=OP.add,
                )
                yield
                yield from squash(t, S1[:], 1, bf16, CFG["newton12"])
                V1 = squash.out
                Dps = pdist.tile([128, CI * CO], f32, tag="D", name=f"D{t}_1")
                yield from d_phase(t, U, V1, Dps, 1)
                # ---- iterations 2, 3 ----
                V = None
                Eprev = None
                HC = CI // 2
                for it in (2, 3):
                    # softmax over co in ci-halves; logits stay in PSUM
                    # (exact f32). Iteration 3 uses exp(D1+D2) =
                    # exp(D1)*exp(D2) so the distance tile only lives
                    # product -> exp.
                    E = statep.tile([128, CI, CO], bf16, tag="E",
                                    name=f"E{t}_{it}")
                    R = statep.tile([128, CI, CO], bf16, tag="R",
                                    name=f"R{t}_{it}")
                    sume = statep.tile([128, CI], f32, tag="sume",
                                       name=f"sume{t}_{it}")
                    rec = statep.tile([128, CI], f32, tag="rec",
                                      name=f"rec{t}_{it}")
                    Lv = Dps[:].rearrange("p (i c) -> p i c", i=CI)
                    nc.scalar.activation(E[:], Lv, AF.Exp)
                    if it == 3:
                        nc.vector.tensor_mul(E[:], E[:], Eprev[:])
                    nc.vector.tensor_reduce(
                        sume[:], E[:], axis=AX.X, op=OP.add
                    )
                    nc.vector.reciprocal(rec[:], sume[:])
                    yield
                    nc.vector.tensor_mul(
                        R[:], E[:],
                        rec[:].unsqueeze(2).broadcast_to([128, CI, CO]),
                    )
                    yield
                    Eprev = E
                    yield from s_phase(t, U, R, it)
                    SB = s_phase.out
                    Sv = SB.rearrange("p (n c) -> p n c", n=NO)
                    if it == 2:
                        yield from squash(t, Sv, it, bf16, CFG["newton12"])
                        V = squash.out
                        Dps = pdist.tile([128, CI * CO], f32, tag="D",
                                         name=f"D{t}_2")
                        yield from d_phase(t, U, V, Dps, 2)
                    else:
                        yield from squash(t, Sv, it, f32, CFG["newton3"])
                        V = squash.out
                emit_out(t, V)

            def drain(gens):
                alive = [g for g in gens if g is not None]
                while alive:
                    for g in list(alive):
                        try:
                            next(g)
                        except StopIteration:
                            alive.remove(g)

            if CFG["pair"]:
                groups = [(0, 1), (2, 3), (4, 5), (6, 7)]
                drain([conv_tile(t) for t in groups[0]])
                for gi, grp in enumerate(groups):
                    gens = [routing_tile(t, *conv_tile.out[t]) for t in grp]
                    if gi + 1 < len(groups):
                        gens += [conv_tile(t) for t in groups[gi + 1]]
                    drain(gens)
            else:
                for t in range(TILES):
                    drain([conv_tile(t)])
                    drain([routing_tile(t, *conv_tile.out[t])])

    nc.compile()
    _BUILT[key] = nc
    return nc


def _assemble(out_halves_all):
    o = out_halves_all.reshape(-1, 2, 4, CO, NPIX)
    return np.ascontiguousarray(
        o.transpose(0, 3, 1, 2, 4).reshape(-1, CO, NO, H, W)
    )


def kernel(x, conv_w, bias):
    import sys
    if "/opt/trn_rl_repo" not in sys.path:
        sys.path.insert(0, "/opt/trn_rl_repo")
    from concourse import bass_utils

    patches, w_m, bias_bc, ident, identb = _host_prep(x, conv_w, bias)
    nc = _build_nc()
    in_maps = [
        {"patches": patches[b], "w": w_m, "bias": bias_bc, "ident": ident,
         "identb": identb}
        for b in range(BS)
    ]
    res = bass_utils.run_bass_kernel_spmd(nc, in_maps, core_ids=list(range(BS)))
    outs = np.stack([r["out"] for r in res.results])
    return _assemble(outs).astype(np.float32)


# revision 20
# speedup vs baseline: 1.0526x; 1.0209x over previous
"""Trainium2 Bass kernel for ConvPixelToCapsules (conv -> 3-iter dynamic routing).

Strategy (hardcoded for x[8,32,8,32,32], conv_w[256,8,3,3], bias[32,8,1,1]):
  - Host precomputes im2col patches per batch element, with an extra 33rd
    "channel" slot holding sum_ci(x) (conv linearity gives iteration-1's
    uniform-route preactivation for free), plus the weight matrix in
    [72, (no,co)] layout and a partition-broadcast bias tile.
  - 8 NeuronCores, data-parallel over batch: core k owns batch element k.
  - Per core: 8 tiles of 128 output pixels. Votes live in SBUF as
    [pixel-partition; (ci,no,co)] bf16; all products are DVE bf16 2x ops.
  - v3: the ci-contraction (preactivation S = sum_ci R*U) runs on the PE as
    32 accumulating identity-matmuls (exact f32 PSUM accumulation, bias
    folded in as a rank-1 matmul), transposed back by the PE. The no-
    contraction (distances) stays a DVE halving tree with the final level
    on GPSIMD. Squash computes sqrt via Quake-rsqrt + Newton on DVE
    (bitcast/shift ops), so the Act engine only ever needs Copy+Exp: one
    activation-table load for the whole program instead of 43.
  - PSUM conv evacuation is paired ([128,512] per copy) to halve Act time.
"""

import numpy as np

BS, CI, NI, H, W = 8, 32, 8, 32, 32
CO, NO = 32, 8
NPIX = H * W            # 1024
TILES = 8               # tiles of 128 pixels per batch element
TP = 128                # pixels per tile (on partitions)
K = 72                  # ni * 3 * 3 contraction
SLOTS = CI + 1          # 32 ci + xsum slot
OUTCH = NO * CO         # 256, (no, co) order
QK = 0x5F3759DF         # Quake rsqrt seed constant

CFG = {
    "pair": True,          # interleave emission of tile pairs
    "newton12": 0,         # Newton iterations for squash 1-2
    "newton3": 2,          # Newton iterations for final squash
    "bias_mm": True,       # fold bias add into the PE accumulation group
    "big_bufs": 3,
    "pconv_bufs": 2,
}

_BUILT = {}


def _host_prep(x, conv_w, bias):
    x = np.asarray(x, np.float32)
    conv_w = np.asarray(conv_w, np.float32)
    bias = np.asarray(bias, np.float32)
    x_pad = np.pad(x, ((0, 0), (0, 0), (0, 0), (1, 1), (1, 1)))
    x_aug = np.concatenate([x_pad, x_pad.sum(1, keepdims=True)], axis=1)
    wv = np.lib.stride_tricks.sliding_window_view(x_aug, (3, 3), axis=(3, 4))
    import ml_dtypes
    cdt_np = ml_dtypes.bfloat16
    patches = np.ascontiguousarray(
        wv.transpose(0, 2, 5, 6, 1, 3, 4).reshape(BS, K, SLOTS, NPIX)
    ).astype(cdt_np)
    w_m = np.ascontiguousarray(
        conv_w.reshape(CO, NO, NI, 3, 3).transpose(2, 3, 4, 1, 0).reshape(K, OUTCH)
    ).astype(cdt_np)
    bias_bc = np.broadcast_to(
        bias[:, :, 0, 0].T.reshape(1, OUTCH), (128, OUTCH)
    ).astype(np.float32)
    ident = np.eye(128, dtype=np.float32)
    identb = np.eye(128, dtype=cdt_np)
    return patches, w_m, bias_bc, ident, identb


def _build_nc():
    key = ("nc",) + tuple(sorted(CFG.items()))
    if key in _BUILT:
        return _BUILT[key]
    import concourse.bacc as bacc
    import concourse.tile as tile
    import concourse.mybir as mybir

    f32 = mybir.dt.float32
    bf16 = mybir.dt.bfloat16
    u32 = mybir.dt.uint32
    AF = mybir.ActivationFunctionType
    OP = mybir.AluOpType
    AX = mybir.AxisListType

    nc = bacc.Bacc("TRN2", target_bir_lowering=False, debug=False, num_devices=8)

    patches_d = nc.dram_tensor("patches", [K, SLOTS, NPIX], bf16, kind="ExternalInput")
    w_d = nc.dram_tensor("w", [K, OUTCH], bf16, kind="ExternalInput")
    bias_d = nc.dram_tensor("bias", [128, OUTCH], f32, kind="ExternalInput")
    ident_d = nc.dram_tensor("ident", [128, 128], f32, kind="ExternalInput")
    identb_d = nc.dram_tensor("identb", [128, 128], bf16, kind="ExternalInput")
    out_d = nc.dram_tensor("out", [2, 128, NPIX], f32, kind="ExternalOutput")

    with tile.TileContext(nc) as tc:
        with (
            tc.tile_pool(name="const", bufs=1) as const,
            tc.tile_pool(name="pat", bufs=3) as patp,
            tc.tile_pool(name="votes", bufs=4) as votesp,
            tc.tile_pool(name="big", bufs=CFG["big_bufs"]) as bigp,
            tc.tile_pool(name="state", bufs=3) as statep,
            tc.tile_pool(name="obuf", bufs=1) as obufp,
            tc.tile_pool(name="pconv", bufs=CFG["pconv_bufs"], space="PSUM") as pconv,
            tc.tile_pool(name="pdist", bufs=2, space="PSUM") as pdist,
            tc.tile_pool(name="ptr", bufs=1, space="PSUM") as ptr,
        ):
            w_sb = const.tile([K, OUTCH], bf16)
            nc.sync.dma_start(w_sb[:], w_d.ap())
            bias_sb = const.tile([128, OUTCH], f32)
            nc.sync.dma_start(bias_sb[:], bias_d.ap())
            ident_sb = const.tile([128, 128], f32)
            nc.sync.dma_start(ident_sb[:], ident_d.ap())
            identb_sb = const.tile([128, 128], bf16)
            nc.sync.dma_start(identb_sb[:], identb_d.ap())
            ones1 = const.tile([1, 128], f32)
            nc.gpsimd.memset(ones1[:], 1.0)
            qc = const.tile([128, CO], u32)
            nc.gpsimd.memset(qc[:], QK)

            ob = [
                obufp.tile([128, NPIX], f32, tag=f"ob{h}", name=f"ob{h}")
                for h in range(2)
            ]

            def conv_tile(t):
                # votes for 128 pixels; Uxs slot first so iteration 1 can
                # start early; ci-pairs share one PSUM bank so the Act
                # evacuation runs half as many, double-width copies.
                pt = patp.tile([K, SLOTS, TP], bf16, tag="pt", name=f"pt{t}")
                nc.sync.dma_start(
                    pt[:, CI, :], patches_d.ap()[:, CI, t * TP : (t + 1) * TP]
                )
                nc.sync.dma_start(
                    pt[:, :CI, :], patches_d.ap()[:, :CI, t * TP : (t + 1) * TP]
                )
                U = votesp.tile([128, CI, NO, CO], bf16, tag="U", name=f"U{t}")
                Uxs = votesp.tile([128, OUTCH], f32, tag="Uxs", name=f"Uxs{t}")
                conv_tile.out[t] = (U, Uxs)
                pvx = pconv.tile([128, 2 * OUTCH], f32, tag="pv", name=f"pvx{t}")
                nc.tensor.matmul(
                    pvx[:, :OUTCH], pt[:, CI, :], w_sb[:], start=True, stop=True
                )
                nc.scalar.copy(Uxs[:], pvx[:, :OUTCH])
                yield
                for c in range(CI // 2):
                    pv = pconv.tile([128, 2 * OUTCH], f32, tag="pv",
                                    name=f"pv{t}_{c}")
                    nc.tensor.matmul(
                        pv[:, :OUTCH], pt[:, 2 * c, :], w_sb[:],
                        start=True, stop=True,
                    )
                    nc.tensor.matmul(
                        pv[:, OUTCH:], pt[:, 2 * c + 1, :], w_sb[:],
                        start=True, stop=True,
                    )
                    dst = U[:, 2 * c : 2 * c + 2].rearrange(
                        "p c n o -> p (c n o)"
                    )
                    nc.scalar.copy(dst, pv[:])
                    yield
            conv_tile.out = {}

            def emit_out(t, V):
                Vf = V[:].rearrange("p n c -> p (n c)")
                for h in range(2):
                    tp = ptr.tile([128, 128], f32, tag="tp", name=f"tp{t}_{h}")
                    nc.tensor.transpose(
                        tp[:], Vf[:, h * 128 : (h + 1) * 128], ident_sb[:]
                    )
                    nc.scalar.copy(ob[h][:, t * TP : (t + 1) * TP], tp[:])
                    nc.sync.dma_start(
                        out_d.ap()[h][:, t * TP : (t + 1) * TP],
                        ob[h][:, t * TP : (t + 1) * TP],
                    )

            def squash(t, S, it, out_dtype, newton):
                # S: [128, NO, CO] f32 (SBUF or PSUM view) -> V [128, NO, CO]
                # scl = sqrt(n)/(1+n) via Quake rsqrt (no act tables needed)
                sq = statep.tile([128, NO, CO], f32, tag="sq", name=f"sq{t}_{it}")
                nc.scalar.square(sq[:], S)
                nsq = statep.tile([128, CO], f32, tag="nsq", name=f"nsq{t}_{it}")
                nc.vector.tensor_reduce(
                    nsq[:], sq[:].transpose([0, 2, 1]), axis=AX.X, op=OP.add
                )
                yield
                sh = statep.tile([128, CO], u32, tag="sh", name=f"sh{t}_{it}")
                nc.vector.tensor_scalar(
                    sh[:], nsq[:].bitcast(u32), 1, None,
                    op0=OP.logical_shift_right,
                )
                y = statep.tile([128, CO], f32, tag="y", name=f"y{t}_{it}")
                nc.vector.tensor_tensor(
                    y[:].bitcast(u32), qc[:], sh[:], op=OP.subtract
                )
                den = statep.tile([128, CO], f32, tag="den", name=f"den{t}_{it}")
                nc.vector.tensor_scalar_add(den[:], nsq[:], 1.0)
                rcd = statep.tile([128, CO], f32, tag="rcd", name=f"rcd{t}_{it}")
                nc.vector.reciprocal(rcd[:], den[:])
                tq = statep.tile([128, CO], f32, tag="tq", name=f"tq{t}_{it}")
                for _ in range(newton):
                    nc.vector.tensor_mul(tq[:], y[:], y[:])
                    nc.vector.tensor_mul(tq[:], tq[:], nsq[:])
                    nc.vector.tensor_scalar(
                        tq[:], tq[:], -0.5, 1.5, op0=OP.mult, op1=OP.add
                    )
                    nc.vector.tensor_mul(y[:], y[:], tq[:])
                yield
                # scl = nsq * y * rcd  (= sqrt(nsq)/(1+nsq))
                scl = statep.tile([128, CO], f32, tag="scl", name=f"scl{t}_{it}")
                nc.vector.tensor_mul(scl[:], nsq[:], y[:])
                nc.vector.tensor_mul(scl[:], scl[:], rcd[:])
                V = statep.tile([128, NO, CO], out_dtype, tag=f"V{it}",
                                name=f"V{t}_{it}")
                nc.vector.tensor_mul(
                    V[:], S, scl[:].unsqueeze(1).broadcast_to([128, NO, CO])
                )
                yield
                squash.out = V

            def s_phase(t, U, R, it):
                # Fused: tmp = U*R (bf16 2x, ci-quarters) pipelined into the
                # PE ci-contraction. Identity stays the stationary, so each
                # matmul is a PSUM-accumulating copy; bias opens the group
                # as a rank-1 ones x bias_row matmul. S lands in [p,(no,co)]
                # f32 PSUM exactly.
                SBt = pdist.tile([128, CI * CO], f32, tag="D",
                                 name=f"SB{t}_{it}")
                SB = SBt[:, :OUTCH]
                tmp = bigp.tile([128, CI, NO, CO], bf16, tag="tmp",
                                name=f"tmps{it}_{t}")
                facb = R[:].unsqueeze(2).broadcast_to([128, CI, NO, CO])
                nc.tensor.matmul(
                    SB, ones1[:], bias_sb[0:1, :],
                    start=True, stop=False, skip_group_check=True,
                )
                Q = CI // 2
                for q in range(2):
                    sl = slice(q * Q, (q + 1) * Q)
                    nc.vector.tensor_mul(tmp[:, sl], U[:, sl], facb[:, sl])
                    yield
                    for ci in range(q * Q, (q + 1) * Q):
                        nc.tensor.matmul(
                            SB, identb_sb[:],
                            tmp[:, ci].rearrange("p n c -> p (n c)"),
                            start=False, stop=(ci == CI - 1),
                            skip_group_check=True,
                        )
                    yield
                s_phase.out = SB

            def d_phase(t, U, V, Dps, it):
                # Fused: tmpn = U*V in no-major layout (strided write keeps
                # co innermost -> DVE 2x survives), pipelined in no-halves
                # into PE accumulating copies: D[p,(ci,co)] = sum_no tmpn.
                # The ci-half h=0 finishes first so the softmax can start
                # on it while h=1 accumulates.
                tmpn = bigp.tile([128, NO, CI, CO], bf16, tag="tmp",
                                 name=f"tmpd{it}_{t}")
                tmp = tmpn[:].transpose([0, 2, 1, 3])
                facb = V[:].unsqueeze(1).broadcast_to([128, CI, NO, CO])
                mvs = [tmpn[:, no].rearrange("p c o -> p (c o)")
                       for no in range(NO)]
                HN = NO // 2
                for g in range(2):
                    nsl = slice(g * HN, (g + 1) * HN)
                    nc.vector.tensor_mul(
                        tmp[:, :, nsl], U[:, :, nsl], facb[:, :, nsl]
                    )
                    yield
                    for no in range(g * HN, (g + 1) * HN):
                        for h in range(2):
                            nc.tensor.matmul(
                                Dps[:, h * 512 : (h + 1) * 512],
                                identb_sb[:],
                                mvs[no][:, h * 512 : (h + 1) * 512],
                                start=(no == 0), stop=(no == NO - 1),
                                skip_group_check=True,
                            )
                        if no % 2 == 1:
                            yield

            def routing_tile(t, U, Uxs):
                # ---- iteration 1: route is uniform 1/CI ----
                S1 = statep.tile([128, NO, CO], f32, tag="S", name=f"S1_{t}")
                nc.vector.scalar_tensor_tensor(
                    S1[:].rearrange("p n c -> p (n c)"), Uxs[:], 1.0 / CI,
                    bias_sb[:], op0=OP.mult, op1=OP.add,
                )
                yield
                yield from squash(t, S1[:], 1, bf16, CFG["newton12"])
                V1 = squash.out
                Dps = pdist.tile([128, CI * CO], f32, tag="D", name=f"D{t}_1")
                yield from d_phase(t, U, V1, Dps, 1)
                # ---- iterations 2, 3 ----
                V = None
                Eprev = None
                HC = CI // 2
                for it in (2, 3):
                    # softmax over co in ci-halves; logits stay in PSUM
                    # (exact f32). Iteration 3 uses exp(D1+D2) =
                    # exp(D1)*exp(D2) so the distance tile only lives
                    # product -> exp.
                    E = statep.tile([128, CI, CO], bf16, tag="E",
                                    name=f"E{t}_{it}")
                    R = statep.tile([128, CI, CO], bf16, tag="R",
                                    name=f"R{t}_{it}")
                    sume = statep.tile([128, CI], f32, tag="sume",
                                       name=f"sume{t}_{it}")
                    rec = statep.tile([128, CI], f32, tag="rec",
                                      name=f"rec{t}_{it}")
                    Lv = Dps[:].rearrange("p (i c) -> p i c", i=CI)
                    nc.scalar.activation(E[:], Lv, AF.Exp)
                    if it == 3:
                        nc.vector.tensor_mul(E[:], E[:], Eprev[:])
                    nc.vector.tensor_reduce(
                        sume[:], E[:], axis=AX.X, op=OP.add
                    )
                    nc.vector.reciprocal(rec[:], sume[:])
                    yield
                    nc.gpsimd.tensor_mul(
                        R[:], E[:],
                        rec[:].unsqueeze(2).broadcast_to([128, CI, CO]),
                    )
                    yield
                    Eprev = E
                    yield from s_phase(t, U, R, it)
                    SB = s_phase.out
                    Sv = SB.rearrange("p (n c) -> p n c", n=NO)
                    if it == 2:
                        yield from squash(t, Sv, it, bf16, CFG["newton12"])
                        V = squash.out
                        Dps = pdist.tile([128, CI * CO], f32, tag="D",
                                         name=f"D{t}_2")
                        yield from d_phase(t, U, V, Dps, 2)
                    else:
                        yield from squash(t, Sv, it, f32, CFG["newton3"])
                        V = squash.out
                emit_out(t, V)

            def drain(gens):
                alive = [g for g in gens if g is not None]
                while alive:
                    for g in list(alive):
                        try:
                            next(g)
                        except StopIteration:
                            alive.remove(g)

            if CFG["pair"]:
                groups = [(0, 1), (2, 3), (4, 5), (6, 7)]
                drain([conv_tile(t) for t in groups[0]])
                for gi, grp in enumerate(groups):
                    gens = [routing_tile(t, *conv_tile.out[t]) for t in grp]
                    if gi + 1 < len(groups):
                        gens += [conv_tile(t) for t in groups[gi + 1]]
                    drain(gens)
            else:
                for t in range(TILES):
                    drain([conv_tile(t)])
                    drain([routing_tile(t, *conv_tile.out[t])])

    nc.compile()
    _BUILT[key] = nc
    return nc


def _assemble(out_halves_all):
    o = out_halves_all.reshape(-1, 2, 4, CO, NPIX)
    return np.ascontiguousarray(
        o.transpose(0, 3, 1, 2, 4).reshape(-1, CO, NO, H, W)
    )


def kernel(x, conv_w, bias):
    import sys
    if "/opt/trn_rl_repo" not in sys.path:
        sys.path.insert(0, "/opt/trn_rl_repo")
    from concourse import bass_utils

    patches, w_m, bias_bc, ident, identb = _host_prep(x, conv_w, bias)
    nc = _build_nc()
    in_maps = [
        {"patches": patches[b], "w": w_m, "bias": bias_bc, "ident": ident,
         "identb": identb}
        for b in range(BS)
    ]
    res = bass_utils.run_bass_kernel_spmd(nc, in_maps, core_ids=list(range(BS)))
    outs = np.stack([r["out"] for r in res.results])
    return _assemble(outs).astype(np.float32)
